# revision 47
# baseline (speedup 1.0000x reference)
"""GAT layer kernel for Trainium2, 8 NeuronCores, data-parallel over batch.

Per batch b (one core each):
    h   = x @ W;  a1 = x @ a[:D];  a2 = x @ a[D:]
    e   = leaky_relu(a1[i] + a2[j], 0.2)
    att = softmax over i of where(adj>0, e, -9e15)
    h'  = att @ h ; out = elu(h') @ han_w + han_b

Attention is computed in transposed [j, i] layout: the softmax reduce is a
free-axis ACT accum_out and att^T feeds the PE as lhsT.  adj stripes are
cast to fp16 "mask + a1[i]" tiles in natural [i, j] layout (a1 rides the
tensor_scalar per-partition operand; masked entries get a1 - 2048 so the
exp underflows to exactly 0) and PE-transposed (transpose mode, fp16 PSUM).
a2[j] rides the Prelu bias.  The softmax division is folded into h
(h_s = h / rowsum); elu's "-1" is folded into b_eff = han_b - sum(han_w).
The big matmul h'^T[d,i] = sum_j hs[j,d] att^T[j,i] accumulates 3/4 fused
inside the attention loop (c0 full + c1 half 0); only c1-half1 sweeps in
the tail, interleaved with the elu + output projection.
"""

import numpy as np

import concourse.bacc as bacc
import concourse.mybir as mybir
from concourse import masks
from concourse.tile import TileContext
from concourse.bass_utils import run_bass_kernel_spmd

P = 128
N = 2048
D = 256
NT = N // P          # 16 node tiles
DC = D // P          # 2 d chunks
NH = N // 2          # i-half size
MASK = 2048.0        # logit offset for masked entries; *0.2 => exp underflows to 0
ALPHA = 0.2

dt = mybir.dt
AF = mybir.ActivationFunctionType
OP = mybir.AluOpType

_CACHED_NC = None


def build_nc():
    nc = bacc.Bacc("TRN2", target_bir_lowering=False, debug=False)

    x_d = nc.dram_tensor("x", [N, D], dt.float32, kind="ExternalInput")
    adj_d = nc.dram_tensor("adj", [N, N], dt.int32, kind="ExternalInput")
    w_d = nc.dram_tensor("W", [D, D], dt.float32, kind="ExternalInput")
    a_d = nc.dram_tensor("a_rs", [D, 2], dt.float32, kind="ExternalInput")
    han_d = nc.dram_tensor("han_w", [D, D], dt.float32, kind="ExternalInput")
    beff_d = nc.dram_tensor("b_eff", [1, D], dt.float32, kind="ExternalInput")
    out_d = nc.dram_tensor("out", [N, D], dt.float32, kind="ExternalOutput")

    # stripe view of adj: adj_r[p, it, j] = adj[it*P + p, j]  (slice j per jt)
    adj_r = adj_d.rearrange("(it p) j -> p it j", p=P)

    with TileContext(nc) as tc:
        with (
            tc.tile_pool(name="const", bufs=1) as cp,
            tc.tile_pool(name="big", bufs=1) as bp,
        ):
            ident16 = cp.tile([P, P], dt.float16)
            masks.make_identity(nc, ident16[:])
            identf = cp.tile([P, P], dt.float32)
            masks.make_identity(nc, identf[:])
            ones1 = cp.tile([1, P], dt.float16)
            nc.vector.memset(ones1[:], 1.0)
            shift = cp.tile([P, 1], dt.float32)
            nc.vector.memset(shift[:], -7.0)
            warm = cp.tile([1, P], dt.float32)
            nc.vector.memset(warm[:], 0.5)
            nc.scalar.activation(warm[:], warm[:], AF.Prelu, alpha=ALPHA)
            nc.scalar.activation(warm[:], warm[:], AF.Exp)

            w_bf = cp.tile([P, DC * D], dt.float16)
            nc.gpsimd.dma_start(w_bf[:].rearrange("p (c d) -> p c d", c=DC),
                                w_d.rearrange("(c p) d -> p c d", p=P))
            han_bf = cp.tile([P, DC * D], dt.float16)
            nc.gpsimd.dma_start(han_bf[:].rearrange("p (c d) -> p c d", c=DC),
                                han_d.rearrange("(c p) d -> p c d", p=P))
            a_f = cp.tile([P, DC * 2], dt.float32)
            nc.sync.dma_start(a_f[:].rearrange("p (c t) -> p c t", c=DC),
                              a_d.rearrange("(c p) t -> p c t", p=P))
            beff_bf = cp.tile([1, D], dt.float16)
            nc.gpsimd.dma_start(beff_bf[:], beff_d[:])

            # persistent per-batch tensors (small per-index tiles to keep
            # dependency tracking fine-grained)
            xT_bf = [bp.tile([P, N], dt.float16, tag=f"xT{c}", name=f"xT{c}")
                     for c in range(DC)]
            c_sb = [bp.tile([P, 1], dt.float32, tag=f"c{i}", name=f"c{i}")
                    for i in range(NT)]                           # a1col - MASK
            a2c_sb = [bp.tile([P, 1], dt.float32, tag=f"a2_{i}", name=f"a2_{i}")
                      for i in range(NT)]                         # a2col
            rs_sb = [bp.tile([P, 1], dt.float32, tag=f"rs{i}", name=f"rs{i}")
                     for i in range(NT)]                          # softmax row sums
            rc_sb = [bp.tile([P, 1], dt.float32, tag=f"rc{i}", name=f"rc{i}")
                     for i in range(NT)]                          # reciprocals
            h_all = [bp.tile([P, D], dt.float32, tag=f"h{i}", name=f"h{i}")
                     for i in range(NT)]                          # h[j, d]
            hs_all = [bp.tile([P, D], dt.float16, tag=f"hs{i}", name=f"hs{i}")
                      for i in range(NT)]                         # h / rowsum
            pt_all = [bp.tile([P, N], dt.float16, tag=f"pt{i}", name=f"pt{i}")
                      for i in range(NT)]                         # att^T

            # ---- stage 1: load x (batched group DMAs), transpose to xT;
            # a1/a2 columns via matmul
            XG = 4  # it-tiles per x DMA group
            with (
                tc.tile_pool(name="xload", bufs=2) as xl,
                tc.tile_pool(name="xtmp", bufs=4) as xt,
                tc.tile_pool(name="xps", bufs=4, space="PSUM") as xps,
                tc.tile_pool(name="aps", bufs=4, space="PSUM") as aps,
            ):
                xxg = None
                for it in range(NT):
                    g, gi = divmod(it, XG)
                    if gi == 0:
                        xxg = xl.tile([P, XG * D], dt.float32, tag="xx", name="xxg")
                        nc.sync.dma_start(
                            xxg[:].rearrange("p (q d) -> p q d", q=XG),
                            x_d.rearrange("(gq p) d -> p gq d",
                                          p=P)[:, g * XG:(g + 1) * XG, :])
                    xx = xxg[:, gi * D:(gi + 1) * D]
                    ac_ps = aps.tile([P, 2], dt.float32, tag="ac")
                    for c in range(DC):
                        tp = xps.tile([P, P], dt.float32, tag="tp")
                        nc.tensor.transpose(tp[:], xx[:, c * P:(c + 1) * P], identf[:])
                        xf = xt.tile([P, P], dt.float32, tag="xf")
                        nc.any.tensor_copy(xf[:], tp[:])
                        nc.any.tensor_copy(xT_bf[c][:, it * P:(it + 1) * P], xf[:])
                        # [a1col a2col] partial: xT_chunk as lhsT, a chunk as rhs
                        nc.tensor.matmul(ac_ps[:], xf[:], a_f[:, c * 2:(c + 1) * 2],
                                         start=(c == 0), stop=(c == DC - 1))
                    nc.vector.tensor_scalar(c_sb[it][:], ac_ps[:, 0:1],
                                            -MASK, None, OP.add)
                    nc.vector.tensor_copy(a2c_sb[it][:], ac_ps[:, 1:2])

            # ---- stage 2: h = x @ W  (bf16 matmul, f32 result)
            with tc.tile_pool(name="hps", bufs=2, space="PSUM") as hps:
                for jt in range(NT):
                    h_ps = hps.tile([P, D], dt.float32, tag="h")
                    for c in range(DC):
                        nc.tensor.matmul(h_ps[:], xT_bf[c][:, jt * P:(jt + 1) * P],
                                         w_bf[:, c * D:(c + 1) * D],
                                         start=(c == 0), stop=(c == DC - 1))
                    nc.any.tensor_copy(h_all[jt][:], h_ps[:])

            # ---- stage 3: attention (transposed) + fused big-mm (c0 full, c1 h0)
            A_bf = [bp.tile([P, N], dt.float16, tag=f"A{c}", name=f"A{c}")
                    for c in range(DC)]

            def elu_part(src, c, off, width, ep_):
                mnneg = ep_.tile([P, width], dt.float16, tag="mn", name="mnneg")
                nc.scalar.activation(mnneg[:], src, AF.Relu, scale=-1.0)
                em = ep_.tile([P, width], dt.float16, tag="em", name="em")
                nc.scalar.activation(em[:], mnneg[:], AF.Exp, scale=-1.0)
                rl = ep_.tile([P, width], dt.float16, tag="rl", name="rl")
                nc.vector.tensor_scalar(rl[:], src, 0.0, None, OP.max)
                nc.vector.tensor_tensor(A_bf[c][:, off:off + width], em[:],
                                        rl[:], OP.add)

            with (
                tc.tile_pool(name="elu", bufs=3) as ep_,
                tc.tile_pool(name="osb", bufs=3) as ob_,
            ):
                with tc.tile_pool(name="htps", bufs=1, space="PSUM") as hp2:
                    hT0 = hp2.tile([P, N], dt.float32, tag="hT0", name="hT0")
                    hT1a = hp2.tile([P, NH], dt.float32, tag="hT1a", name="hT1a")
                    with (
                        tc.tile_pool(name="adjs", bufs=4) as ap_,
                        tc.tile_pool(name="adjm", bufs=4) as am_,
                        tc.tile_pool(name="lrl", bufs=3) as lp_,
                        tc.tile_pool(name="pre", bufs=2, space="PSUM") as pp_,
                    ):
                        for jt in range(NT):
                            adjs = ap_.tile([P, N], dt.int32, tag="adjs")
                            nc.sync.dma_start(
                                adjs[:].rearrange("p (it jj) -> p it jj", jj=P),
                                adj_r[:, :, jt * P:(jt + 1) * P])
                            adjm = am_.tile([P, N], dt.float16, tag="adjm")
                            for it in range(NT):
                                sl = slice(it * P, (it + 1) * P)
                                eng = nc.gpsimd if it % 3 != 2 else nc.vector
                                eng.tensor_scalar(adjm[:, sl], adjs[:, sl],
                                                  MASK, c_sb[it][:],
                                                  OP.mult, OP.add)
                            lrl = lp_.tile([P, N], dt.float16, tag="lrl",
                                           name="lrl")
                            for hf in range(2):
                                pre = pp_.tile([P, NH], dt.float16, tag="pre",
                                               name="pre")
                                for ii in range(NH // P):
                                    it = hf * (NH // P) + ii
                                    nc.tensor.transpose(
                                        pre[:, ii * P:(ii + 1) * P],
                                        adjm[:, it * P:(it + 1) * P], ident16[:])
                                hsl = slice(hf * NH, (hf + 1) * NH)
                                if (2 * jt + hf) % 2 == 0:
                                    nc.scalar.activation(lrl[:, hsl], pre[:],
                                                         AF.Prelu,
                                                         bias=a2c_sb[jt][:],
                                                         scale=1.0, alpha=ALPHA)
                                else:
                                    z2 = lp_.tile([P, NH], dt.float16, tag="z2",
                                                  name="z2")
                                    nc.vector.tensor_scalar(z2[:], pre[:],
                                                            a2c_sb[jt][:], None,
                                                            OP.add)
                                    z1 = lp_.tile([P, NH], dt.float16, tag="z1",
                                                  name="z1")
                                    nc.vector.tensor_scalar(z1[:], z2[:], ALPHA,
                                                            None, OP.mult)
                                    nc.vector.tensor_tensor(lrl[:, hsl], z2[:],
                                                            z1[:], OP.max)
                            nc.scalar.activation(pt_all[jt][:], lrl[:],
                                                 AF.Exp, bias=shift[:],
                                                 accum_out=rs_sb[jt][:])
                            nc.vector.reciprocal(rc_sb[jt][:], rs_sb[jt][:])
                            nc.vector.tensor_scalar(hs_all[jt][:], h_all[jt][:],
                                                    rc_sb[jt][:], None, OP.mult)
                            # fused big-mm: c0 over full i, c1 over half 0
                            for nb in range(N // 512):
                                nc.tensor.matmul(
                                    hT0[:, nb * 512:(nb + 1) * 512],
                                    hs_all[jt][:, 0:P],
                                    pt_all[jt][:, nb * 512:(nb + 1) * 512],
                                    start=(jt == 0), stop=(jt == NT - 1))
                            for nb in range(NH // 512):
                                nc.tensor.matmul(
                                    hT1a[:, nb * 512:(nb + 1) * 512],
                                    hs_all[jt][:, P:2 * P],
                                    pt_all[jt][:, nb * 512:(nb + 1) * 512],
                                    start=(jt == 0), stop=(jt == NT - 1))

                    # pre pool is closed; its 2 banks host the out-proj psum.
                    with tc.tile_pool(name="ops", bufs=2, space="PSUM") as op_:
                        def out_tile(it0):
                            # two node-tiles per psum/copy/DMA to cut tail dribble
                            o_ps = op_.tile([P, 2 * D], dt.float32, tag="o",
                                            name="o_ps")
                            for k in range(2):
                                it = it0 + k
                                osl = slice(k * D, (k + 1) * D)
                                for c in range(DC):
                                    nc.tensor.matmul(
                                        o_ps[:, osl],
                                        A_bf[c][:, it * P:(it + 1) * P],
                                        han_bf[:, c * D:(c + 1) * D],
                                        start=(c == 0), stop=False)
                                nc.tensor.matmul(o_ps[:, osl], ones1[:], beff_bf[:],
                                                 start=False, stop=True)
                            o_sb = ob_.tile([P, 2 * D], dt.float32, tag="o",
                                            name="o_sb")
                            nc.any.tensor_copy(o_sb[:], o_ps[:])
                            nc.sync.dma_start(
                                out_d.rearrange("(q p) d -> p q d",
                                                p=P)[:, it0:it0 + 2, :],
                                o_sb[:].rearrange("p (q d) -> p q d", q=2))

                        elu_part(hT1a[:], 1, 0, NH, ep_)
                        elu_part(hT0[:, 0:NH], 0, 0, NH, ep_)
                        hT1b = hp2.tile([P, NH], dt.float32, tag="hT1a",
                                        name="hT1b")
                        for jt in range(NT):
                            for nb in range(NH // 512):
                                nc.tensor.matmul(
                                    hT1b[:, nb * 512:(nb + 1) * 512],
                                    hs_all[jt][:, P:2 * P],
                                    pt_all[jt][:, NH + nb * 512:
                                           NH + (nb + 1) * 512],
                                    start=(jt == 0), stop=(jt == NT - 1))
                            if jt % 4 == 3 and (jt // 4) * 2 < NT // 2:
                                out_tile((jt // 4) * 2)
                        elu_part(hT0[:, NH:N], 0, NH, NH, ep_)
                        elu_part(hT1b[:], 1, NH, NH, ep_)
                        for it in range(NT // 2, NT, 2):
                            out_tile(it)

    nc.compile()
    return nc


def _get_nc():
    global _CACHED_NC
    if _CACHED_NC is None:
        _CACHED_NC = build_nc()
    return _CACHED_NC


def run(inputs, trace=False):
    x = np.asarray(inputs["x"], dtype=np.float32)
    adj = np.asarray(inputs["adj"], dtype=np.int32)
    W = np.asarray(inputs["W"], dtype=np.float32)
    a = np.asarray(inputs["a"], dtype=np.float32)
    han_w = np.asarray(inputs["han_w"], dtype=np.float32)
    han_b = np.asarray(inputs["han_b"], dtype=np.float32)

    B = x.shape[0]
    a_rs = np.ascontiguousarray(a.reshape(2, D).T)          # [D, 2]
    b_eff = (han_b - han_w.sum(axis=0)).reshape(1, D)       # elu "-1" folded in

    nc = _get_nc()
    in_maps = [
        {
            "x": np.ascontiguousarray(x[b]),
            "adj": np.ascontiguousarray(adj[b]),
            "W": W,
            "a_rs": a_rs,
            "han_w": han_w,
            "b_eff": b_eff,
        }
        for b in range(B)
    ]
    last_err = None
    for attempt in range(3):
        try:
            res = run_bass_kernel_spmd(nc, in_maps, core_ids=list(range(B)),
                                       trace=trace)
            out = np.stack([np.asarray(r["out"]) for r in res.results], axis=0)
            return out, res
        except Exception as e:  # transient NRT/axon execute failures
            last_err = e
            import time as _time
            _time.sleep(3.0 + 5.0 * attempt)
    raise last_err


def kernel(**inputs) -> np.ndarray:
    out, _ = run(inputs, trace=False)
    return out
